# revision 20
# baseline (speedup 1.0000x reference)
"""Bass/Tile TRN2 kernel for a 2-layer Bayesian LSTM + MLP head.

Contract: kernel(**inputs) takes the FULL unsharded inputs (np arrays, keyed
as in setup_inputs()) and returns the FULL [8192] fp32 output.

Strategy: pure data-parallel over 8 NeuronCores — batch 8192 -> 1024/core,
all (small) weights replicated; the recurrence is local per shard.

On-device design (per core, B=1024):
  - Feature-major layout everywhere: tensors are [feature partitions, batch].
  - Weight sampling (mu + softplus(rho) * eps) done on device (tiny).
  - Pre-pass: transpose x [1024, 2400] -> xT [2400, 1024] in DRAM via PE
    transposes, so per-step x slices load as contiguous feature-major tiles.
  - L1 (H=64): two 512-batch halves packed on 128 partitions. Gates are
    computed straight into PSUM; the input projection, hidden projection and
    bias all accumulate in one PSUM group per gate tile (x rows + ones row
    are concatenated under h in the rhs tile, K=89 one-shot for half A;
    half B runs split MMs at partition bases 64/0 due to the tile_position
    legality rules). Sigmoid over all three sigmoid-gates in ONE ACT op on a
    [128, 1536] PSUM tile; tanh(g) / tanh(c) separate; cell update on DVE
    with the i*g~ product offloaded to GPSIMD.
  - h1 staged to DRAM; L2 (H2=128) runs the same scheme with 2 batch chunks
    and K=65 aux matmuls (h1 + ones row) + K=128 recurrent matmuls.
  - Head: tiny K=128/8 matmuls + Relu-with-bias ACT ops.
"""

import sys

import numpy as np

_REPO = "/opt/trn_rl_repo"
if _REPO not in sys.path:
    sys.path.insert(0, _REPO)

import concourse.bass as bass
import concourse.tile as tile
from concourse import bacc, mybir
from concourse.bass_utils import run_bass_kernel_spmd

F32 = mybir.dt.float32
BF16 = mybir.dt.bfloat16
AF = mybir.ActivationFunctionType

NCORES = 8
B, T, I, H, N = 8192, 100, 24, 64, 8
BC = B // NCORES  # 1024 batch per core
BH = BC // 2      # 512 half-batch
H2 = 2 * H        # 128
G1 = 4 * H        # 256
G2 = 4 * H2       # 512
TI = T * I        # 2400

PARAMS = [
    ("l1_wih", (I, G1)), ("l1_whh", (H, G1)), ("l1_b", (G1,)),
    ("l2_wih", (H, G2)), ("l2_whh", (H2, G2)), ("l2_b", (G2,)),
    ("fc1_w", (N, H2)), ("fc1_b", (N,)),
    ("fc2_w", (N, N)), ("fc2_b", (N,)),
    ("out_w", (1, N)), ("out_b", (1,)),
]

# gate column order in the 4H axis is i, f, g, o. The sigmoid PSUM tile packs
# [i | f | o] along free dim; g gets its own tile (tanh).


def _build(t_steps=T):
    # Bacc (not raw Bass): its finalize() runs the TRN2 legalization passes
    # (sync-wait splitting via event semaphores, nop fusion, etc.)
    nc = bacc.Bacc()

    TIl = t_steps * I
    x = nc.dram_tensor("x", [BC, t_steps, I], F32, kind="ExternalInput")
    prm = {}
    for name, _shape in PARAMS:
        for sfx in ("mu", "rho", "eps"):
            n = f"{name}_{sfx}"
            prm[n] = nc.dram_tensor(n, list(_shape), F32, kind="ExternalInput")
    y = nc.dram_tensor("y", [BC], F32, kind="ExternalOutput")
    xT = nc.dram_tensor("xT", [TIl, BC], BF16)          # transposed input (bf16)

    with tile.TileContext(nc) as tc:
        _frees = []  # keep pool-free closures alive; released at ctx exit

        def fixed(shape, name, dtype=F32):
            t, free = tc.tile(shape, dtype, name=name)
            _frees.append(free)
            return t

        # ---------------- persistent weight tiles ----------------
        W1A = fixed([128, G1], "W1A", BF16)    # 0:64 whh1, 64:88 w1i, 88 b1
        W1hB = fixed([128, G1], "W1hB", BF16)  # 64:128 whh1
        W1xB = fixed([32, G1], "W1xB", BF16)   # 0:24 w1i, 24 b1
        W2h = fixed([128, G2], "W2h", BF16)    # 0:128 whh2
        W2x = fixed([128, G2], "W2x", BF16)    # 0:64 w2i, 64 b2
        fc1wT = fixed([128, N], "fc1wT", BF16)
        fc2wT = fixed([N, N], "fc2wT", BF16)
        outwT = fixed([N, 1], "outwT", BF16)
        fc1b = fixed([N, 1], "fc1b")
        fc2b = fixed([N, 1], "fc2b")
        outb = fixed([1, 1], "outb")
        ident = fixed([128, 128], "ident")
        b1s = fixed([1, G1], "b1s", BF16)

        from concourse.masks import make_identity
        make_identity(nc, ident[:, :])

        # ---------------- sample weights: w = mu + softplus(rho) * eps ------
        with tc.tile_pool(name="wload", bufs=2) as wl:
            def sample(pname, apfn, P, Fr, pbase, dst):
                sl = slice(pbase, pbase + P)
                mu = wl.tile([128, Fr], F32, tag="smu", name="smu")
                rho = wl.tile([128, Fr], F32, tag="srho", name="srho")
                eps = wl.tile([128, Fr], F32, tag="seps", name="seps")
                nc.sync.dma_start(out=mu[sl, :], in_=apfn(prm[f"{pname}_mu"]))
                nc.sync.dma_start(out=rho[sl, :], in_=apfn(prm[f"{pname}_rho"]))
                nc.sync.dma_start(out=eps[sl, :], in_=apfn(prm[f"{pname}_eps"]))
                # softplus(rho) = ln(1 + exp(rho)) via Exp then Ln(x + 1)
                nc.scalar.activation(rho[sl, :], rho[sl, :], AF.Exp)
                nc.scalar.activation(rho[sl, :], rho[sl, :], AF.Ln, bias=1.0)
                nc.vector.tensor_mul(rho[sl, :], rho[sl, :], eps[sl, :])
                nc.vector.tensor_add(dst, rho[sl, :], mu[sl, :])

            id2 = lambda h: h[:, :]
            row = lambda h: h[:].rearrange("(a f) -> a f", a=1)
            col = lambda h: h[:].rearrange("(f a) -> f a", a=1)
            tr2 = lambda h: h[:, :].rearrange("n k -> k n")

            sample("l1_whh", id2, H, G1, 0, W1A[0:H, :])
            sample("l1_wih", id2, I, G1, H, W1A[H:H + I, :])
            # biases must be sampled at a 32-aligned partition base, then
            # DMA-copied into their (unaligned) weight-tile rows.
            sample("l1_b", row, 1, G1, 0, b1s[0:1, :])
            nc.sync.dma_start(out=W1A[H + I:H + I + 1, :], in_=b1s[0:1, :])
            nc.sync.dma_start(out=W1xB[I:I + 1, :], in_=b1s[0:1, :])
            sample("l1_whh", id2, H, G1, 64, W1hB[64:128, :])
            sample("l1_wih", id2, I, G1, 0, W1xB[0:I, :])
            sample("l2_whh", id2, H2, G2, 0, W2h[:, :])
            sample("l2_wih", id2, H, G2, 0, W2x[0:H, :])
            sample("l2_b", row, 1, G2, H, W2x[H:H + 1, :])
            sample("fc1_w", tr2, H2, N, 0, fc1wT[:, :])
            sample("fc2_w", tr2, N, N, 0, fc2wT[:, :])
            sample("out_w", tr2, N, 1, 0, outwT[:, :])
            sample("fc1_b", col, N, 1, 0, fc1b[:, :])
            sample("fc2_b", col, N, 1, 0, fc2b[:, :])
            sample("out_b", col, 1, 1, 0, outb[:, :])

        # ---------------- pre-pass: xT = x.T via PE transposes --------------
        # keep all batch tiles resident; assemble whole [128, BC] row-blocks
        # in SBUF so each xT write is one big contiguous DMA.
        NBLK = (TIl + 127) // 128
        NBT = BC // 128
        with tc.tile_pool(name="xload", bufs=1) as xl, \
             tc.tile_pool(name="xst", bufs=2) as xs, \
             tc.tile_pool(name="xps", bufs=4, space="PSUM") as xp:
            xins = []
            for bt in range(NBT):
                xin = xl.tile([128, TIl], F32, tag=f"xin{bt}", name=f"xin{bt}")
                nc.sync.dma_start(
                    out=xin[:, :],
                    in_=x[bt * 128:(bt + 1) * 128, :, :].rearrange("b t i -> b (t i)"),
                )
                xins.append(xin)
            for blk in range(NBLK):
                w = min(128, TIl - blk * 128)
                stg = xs.tile([128, BC], BF16, tag="stg", name="stg")
                for bt in range(NBT):
                    ps = xp.tile([128, 128], F32, tag="tps", name="tps")
                    nc.tensor.transpose(
                        ps[0:w, 0:128],
                        xins[bt][:, blk * 128:blk * 128 + w], ident[:, :]
                    )
                    if bt % 2 == 0:
                        nc.vector.tensor_copy(
                            stg[0:w, bt * 128:(bt + 1) * 128], ps[0:w, :])
                    else:
                        nc.scalar.copy(
                            stg[0:w, bt * 128:(bt + 1) * 128], ps[0:w, :])
                nc.sync.dma_start(out=xT[blk * 128:blk * 128 + w, :],
                                  in_=stg[0:w, :])

        tc.strict_bb_all_engine_barrier()

        # -------- fused recurrence: L1 step u + L2 step u-1 per iteration ----
        # hxA: rows 0:64 h1(batch half A), 64:88 x_t, 88 ones  (rhs K=89 @ base 0)
        # hxB: rows 0:24 x_t, 24 ones, 64:128 h1(batch half B)
        # L2 runs one step behind L1; h1_t is copied (SBUF->SBUF DMA) into the
        # aux tiles ([h1; ones], K=65 rhs) the same iteration it is produced.
        hxA = [fixed([128, BH], f"hxA{k}", BF16) for k in range(2)]
        hxB = [fixed([128, BH], f"hxB{k}", BF16) for k in range(2)]
        c1t = fixed([128, BH], "c1t")
        ones_row = fixed([1, BH], "ones_row", BF16)
        h2 = [fixed([128, BH], f"h2_{ch}", BF16) for ch in range(2)]
        c2 = [fixed([128, BH], f"c2_{ch}") for ch in range(2)]
        aux = [[fixed([128, BH], f"aux{ch}_{k}", BF16) for k in range(2)]
               for ch in range(2)]
        nc.vector.memset(ones_row[:, :], 1.0)
        nc.vector.memset(c1t[:, :], 0.0)
        nc.vector.memset(hxA[0][0:H, :], 0.0)
        nc.vector.memset(hxB[0][64:128, :], 0.0)
        for k in range(2):
            # ones rows sit at unaligned partitions -> fill via DMA copy
            nc.sync.dma_start(out=hxA[k][H + I:H + I + 1, :], in_=ones_row[0:1, :])
            nc.sync.dma_start(out=hxB[k][I:I + 1, :], in_=ones_row[0:1, :])
        for ch in range(2):
            nc.vector.memset(h2[ch][:, :], 0.0)
            nc.vector.memset(c2[ch][:, :], 0.0)
            for k in range(2):
                nc.vector.memset(aux[ch][k][H:H + 1, :], 1.0)

        # (sigma-free-offset, weight-col-offset): i, f, o then g
        L1_SIG = [(0, 0), (BH, H), (2 * BH, 3 * H)]
        L1_G = 2 * H
        L2_SIG = [(0, 0), (BH, H2), (2 * BH, 3 * H2)]
        L2_G = 2 * H2

        with tc.tile_pool(name="p1ps", bufs=1, space="PSUM") as pps, \
             tc.tile_pool(name="p1sb", bufs=2) as psb, \
             tc.tile_pool(name="p2ps", bufs=1, space="PSUM") as pps2, \
             tc.tile_pool(name="p2sb", bufs=2) as psb2:

            def l1_step(t):
                cur, nxt = t % 2, (t + 1) % 2
                nc.sync.dma_start(out=hxA[cur][H:H + I, :],
                                  in_=xT[t * I:(t + 1) * I, 0:BH])
                nc.sync.dma_start(out=hxB[cur][0:I, :],
                                  in_=xT[t * I:(t + 1) * I, BH:BC])
                sps = pps.tile([128, 3 * BH], F32, tag="sps", name="sps")
                gps = pps.tile([128, BH], F32, tag="gps", name="gps")
                for fo, wc in L1_SIG + [(None, L1_G)]:
                    wsl = slice(wc, wc + H)
                    if fo is None:
                        outA, outB = gps[0:64, :], gps[64:128, :]
                    else:
                        outA = sps[0:64, fo:fo + BH]
                        outB = sps[64:128, fo:fo + BH]
                    nc.tensor.matmul(outA, lhsT=W1A[0:H + I + 1, wsl],
                                     rhs=hxA[cur][0:H + I + 1, :],
                                     start=True, stop=True)
                    nc.tensor.matmul(outB, lhsT=W1hB[64:128, wsl],
                                     rhs=hxB[cur][64:128, :],
                                     start=True, stop=False)
                    nc.tensor.matmul(outB, lhsT=W1xB[0:I + 1, wsl],
                                     rhs=hxB[cur][0:I + 1, :],
                                     start=False, stop=True)
                ssb = psb.tile([128, 3 * BH], F32, tag="ssb", name="ssb")
                tg = psb.tile([128, BH], F32, tag="tg", name="tg")
                tcn = psb.tile([128, BH], F32, tag="tcn", name="tcn")
                pp = psb.tile([128, BH], F32, tag="pp", name="pp")
                qq = psb.tile([128, BH], F32, tag="qq", name="qq")
                nc.scalar.activation(ssb[:, :], sps[:, :], AF.Sigmoid)
                nc.scalar.activation(tg[:, :], gps[:, :], AF.Tanh)
                nc.vector.tensor_mul(pp[:, :], ssb[:, BH:2 * BH], c1t[:, :])
                nc.gpsimd.tensor_mul(qq[:, :], ssb[:, 0:BH], tg[:, :])
                nc.vector.tensor_add(c1t[:, :], pp[:, :], qq[:, :])
                nc.scalar.activation(tcn[:, :], c1t[:, :], AF.Tanh)
                nc.vector.tensor_mul(hxA[nxt][0:H, :],
                                     ssb[0:H, 2 * BH:3 * BH], tcn[0:H, :])
                nc.vector.tensor_mul(hxB[nxt][64:128, :],
                                     ssb[64:128, 2 * BH:3 * BH], tcn[64:128, :])
                # hand h1_t to layer 2 (partition-shifting copies -> DMA)
                nc.sync.dma_start(out=aux[0][t % 2][0:H, :], in_=hxA[nxt][0:H, :])
                nc.sync.dma_start(out=aux[1][t % 2][0:H, :],
                                  in_=hxB[nxt][64:128, :])

            def l2_step(t):
                k = t % 2
                for ch in range(2):
                    sps = pps2.tile([128, 3 * BH], F32, tag="sps2", name="sps2")
                    gps = pps2.tile([128, BH], F32, tag="gps2", name="gps2")
                    for fo, wc in L2_SIG + [(None, L2_G)]:
                        wsl = slice(wc, wc + H2)
                        out = gps[:, :] if fo is None else sps[:, fo:fo + BH]
                        nc.tensor.matmul(out, lhsT=W2x[0:H + 1, wsl],
                                         rhs=aux[ch][k][0:H + 1, :],
                                         start=True, stop=False)
                        nc.tensor.matmul(out, lhsT=W2h[:, wsl],
                                         rhs=h2[ch][:, :],
                                         start=False, stop=True)
                    ssb = psb2.tile([128, 3 * BH], F32, tag="ssb2", name="ssb2")
                    tg = psb2.tile([128, BH], F32, tag="tg2", name="tg2")
                    tcn = psb2.tile([128, BH], F32, tag="tcn2", name="tcn2")
                    pp = psb2.tile([128, BH], F32, tag="pp2", name="pp2")
                    qq = psb2.tile([128, BH], F32, tag="qq2", name="qq2")
                    nc.scalar.activation(ssb[:, :], sps[:, :], AF.Sigmoid)
                    nc.scalar.activation(tg[:, :], gps[:, :], AF.Tanh)
                    nc.vector.tensor_mul(pp[:, :], ssb[:, BH:2 * BH], c2[ch][:, :])
                    nc.gpsimd.tensor_mul(qq[:, :], ssb[:, 0:BH], tg[:, :])
                    nc.vector.tensor_add(c2[ch][:, :], pp[:, :], qq[:, :])
                    nc.scalar.activation(tcn[:, :], c2[ch][:, :], AF.Tanh)
                    nc.vector.tensor_mul(h2[ch][:, :],
                                         ssb[:, 2 * BH:3 * BH], tcn[:, :])

            for u in range(t_steps + 1):
                if u < t_steps:
                    l1_step(u)
                if u >= 1:
                    l2_step(u - 1)

        # ---------------- head: fc1 -> relu -> fc2 -> relu -> out -----------
        with tc.tile_pool(name="hps", bufs=2, space="PSUM") as hps, \
             tc.tile_pool(name="hsb", bufs=2) as hsb:
            for ch in range(2):
                f1 = hps.tile([N, BH], F32, tag="f1", name="f1")
                nc.tensor.matmul(f1[0:N, :], lhsT=fc1wT[0:H2, 0:N],
                                 rhs=h2[ch][:, :], start=True, stop=True)
                x1 = hsb.tile([N, BH], BF16, tag="x1", name="x1")
                nc.scalar.activation(x1[0:N, :], f1[0:N, :], AF.Relu,
                                     bias=fc1b[:, :])
                f2 = hps.tile([N, BH], F32, tag="f2", name="f2")
                nc.tensor.matmul(f2[0:N, :], lhsT=fc2wT[0:N, 0:N],
                                 rhs=x1[0:N, :], start=True, stop=True)
                x2 = hsb.tile([N, BH], BF16, tag="x2", name="x2")
                nc.scalar.activation(x2[0:N, :], f2[0:N, :], AF.Relu,
                                     bias=fc2b[:, :])
                fy = hps.tile([1, BH], F32, tag="fy", name="fy")
                nc.tensor.matmul(fy[0:1, :], lhsT=outwT[0:N, 0:1],
                                 rhs=x2[0:N, :], start=True, stop=True)
                ysb = hsb.tile([1, BH], F32, tag="ysb", name="ysb")
                nc.scalar.activation(ysb[0:1, :], fy[0:1, :], AF.Identity,
                                     bias=outb[:, :])
                nc.sync.dma_start(
                    out=y[ch * BH:(ch + 1) * BH].rearrange("(a f) -> a f", a=1),
                    in_=ysb[0:1, :],
                )

        # release single-tile pools in LIFO order so no pool-boundary
        # pseudo-instructions survive into the lowered BIR
        for free in reversed(_frees):
            free()

    # run the bacc legalization pipeline (sync-wait splitting, reg alloc, ...)
    nc.finalize()
    return nc


def run(inputs, trace=False):
    """Returns (y_full [8192] f32, BassKernelResults)."""
    xfull = np.ascontiguousarray(np.asarray(inputs["input_seq"], dtype=np.float32))
    base = {}
    for name, _shape in PARAMS:
        for sfx in ("mu", "rho", "eps"):
            n = f"{name}_{sfx}"
            base[n] = np.ascontiguousarray(np.asarray(inputs[n], dtype=np.float32))
    in_maps = []
    for c in range(NCORES):
        m = dict(base)
        m["x"] = np.ascontiguousarray(xfull[c * BC:(c + 1) * BC])
        in_maps.append(m)
    nc = _build()
    res = run_bass_kernel_spmd(nc, in_maps, core_ids=list(range(NCORES)),
                               trace=trace)
    out = np.concatenate([r["y"] for r in res.results]).astype(np.float32)
    return out, res


def kernel(**inputs):
    out, _ = run(inputs, trace=False)
    return out


# revision 30
# speedup vs baseline: 105.9814x; 105.9814x over previous
"""Bass/Tile TRN2 kernel for a 2-layer Bayesian LSTM + MLP head.

Contract: kernel(**inputs) takes the FULL unsharded inputs (np arrays, keyed
as in setup_inputs()) and returns the FULL [8192] fp32 output.

Strategy: pure data-parallel over 8 NeuronCores — batch 8192 -> 1024/core,
all (small) weights replicated; the recurrence is local per shard.

On-device design (per core, B=1024):
  - Feature-major layout everywhere: tensors are [feature partitions, batch].
  - Weight sampling (mu + softplus(rho) * eps) done on device (tiny).
  - Pre-pass: transpose x [1024, 2400] -> xT [2400, 1024] in DRAM via PE
    transposes, so per-step x slices load as contiguous feature-major tiles.
  - L1 (H=64): two 512-batch halves packed on 128 partitions. Gates are
    computed straight into PSUM; the input projection, hidden projection and
    bias all accumulate in one PSUM group per gate tile (x rows + ones row
    are concatenated under h in the rhs tile, K=89 one-shot for half A;
    half B runs split MMs at partition bases 64/0 due to the tile_position
    legality rules). Sigmoid over all three sigmoid-gates in ONE ACT op on a
    [128, 1536] PSUM tile; tanh(g) / tanh(c) separate; cell update on DVE
    with the i*g~ product offloaded to GPSIMD.
  - h1 staged to DRAM; L2 (H2=128) runs the same scheme with 2 batch chunks
    and K=65 aux matmuls (h1 + ones row) + K=128 recurrent matmuls.
  - Head: tiny K=128/8 matmuls + Relu-with-bias ACT ops.
"""

import sys

import numpy as np

_REPO = "/opt/trn_rl_repo"
if _REPO not in sys.path:
    sys.path.insert(0, _REPO)

import concourse.bass as bass
import concourse.tile as tile
from concourse import bacc, mybir
from concourse.bass_utils import run_bass_kernel_spmd

F32 = mybir.dt.float32
BF16 = mybir.dt.bfloat16
AF = mybir.ActivationFunctionType

NCORES = 8
B, T, I, H, N = 8192, 100, 24, 64, 8
BC = B // NCORES  # 1024 batch per core
BH = BC // 2      # 512 half-batch
H2 = 2 * H        # 128
G1 = 4 * H        # 256
G2 = 4 * H2       # 512
TI = T * I        # 2400

PARAMS = [
    ("l1_wih", (I, G1)), ("l1_whh", (H, G1)), ("l1_b", (G1,)),
    ("l2_wih", (H, G2)), ("l2_whh", (H2, G2)), ("l2_b", (G2,)),
    ("fc1_w", (N, H2)), ("fc1_b", (N,)),
    ("fc2_w", (N, N)), ("fc2_b", (N,)),
    ("out_w", (1, N)), ("out_b", (1,)),
]

# gate column order in the 4H axis is i, f, g, o. The sigmoid PSUM tile packs
# [i | f | o] along free dim; g gets its own tile (tanh).


def _build(t_steps=T):
    # Bacc (not raw Bass): its finalize() runs the TRN2 legalization passes
    # (sync-wait splitting via event semaphores, nop fusion, etc.)
    nc = bacc.Bacc()

    TIl = t_steps * I
    x = nc.dram_tensor("x", [BC, t_steps, I], F32, kind="ExternalInput")
    prm = {}
    for name, _shape in PARAMS:
        for sfx in ("mu", "rho", "eps"):
            n = f"{name}_{sfx}"
            prm[n] = nc.dram_tensor(n, list(_shape), F32, kind="ExternalInput")
    y = nc.dram_tensor("y", [BC], F32, kind="ExternalOutput")
    xT = nc.dram_tensor("xT", [TIl, BC], BF16)          # transposed input (bf16)

    with tile.TileContext(nc) as tc:
        _frees = []  # keep pool-free closures alive; released at ctx exit

        def fixed(shape, name, dtype=F32):
            t, free = tc.tile(shape, dtype, name=name)
            _frees.append(free)
            return t

        # ---------------- persistent weight tiles ----------------
        W1A = fixed([128, G1], "W1A", BF16)    # 0:64 whh1, 64:88 w1i, 88 b1
        W1hB = fixed([128, G1], "W1hB", BF16)  # 64:128 whh1
        W1xB = fixed([32, G1], "W1xB", BF16)   # 0:24 w1i, 24 b1
        W2h = fixed([128, G2], "W2h", BF16)    # 0:128 whh2
        W2x = fixed([128, G2], "W2x", BF16)    # 0:64 w2i, 64 b2
        fc1wT = fixed([128, N], "fc1wT", BF16)
        fc2wT = fixed([N, N], "fc2wT", BF16)
        outwT = fixed([N, 1], "outwT", BF16)
        fc1b = fixed([N, 1], "fc1b")
        fc2b = fixed([N, 1], "fc2b")
        outb = fixed([1, 1], "outb")
        ident = fixed([128, 128], "ident")
        b1s = fixed([1, G1], "b1s", BF16)

        from concourse.masks import make_identity
        make_identity(nc, ident[:, :])

        # ---------------- sample weights: w = mu + softplus(rho) * eps ------
        # two passes so all Exp ops run together, then all Ln ops — avoids
        # per-param activation-table reloads (~1.3 us each).
        with tc.tile_pool(name="wload", bufs=1) as wl:
            _fin = []

            def sample(pname, apfn, P, Fr, pbase, dst):
                sl = slice(pbase, pbase + P)
                mu = wl.tile([128, Fr], F32, tag=f"smu{len(_fin)}", name="smu")
                rho = wl.tile([128, Fr], F32, tag=f"srho{len(_fin)}", name="srho")
                eps = wl.tile([128, Fr], F32, tag=f"seps{len(_fin)}", name="seps")
                nc.sync.dma_start(out=mu[sl, :], in_=apfn(prm[f"{pname}_mu"]))
                nc.sync.dma_start(out=rho[sl, :], in_=apfn(prm[f"{pname}_rho"]))
                nc.sync.dma_start(out=eps[sl, :], in_=apfn(prm[f"{pname}_eps"]))
                # softplus(rho) = ln(1 + exp(rho)) via Exp then Ln(x + 1)
                nc.scalar.activation(rho[sl, :], rho[sl, :], AF.Exp)
                _fin.append((mu, rho, eps, sl, dst))

            def finish_samples():
                # keep all Exps strictly before all Lns so the ACT table set
                # switches once, not per-param (scheduler-only fence)
                tc.no_sync_barrier()
                for mu, rho, eps, sl, dst in _fin:
                    nc.scalar.activation(rho[sl, :], rho[sl, :], AF.Ln, bias=1.0)
                for mu, rho, eps, sl, dst in _fin:
                    nc.vector.tensor_mul(rho[sl, :], rho[sl, :], eps[sl, :])
                    nc.vector.tensor_add(dst, rho[sl, :], mu[sl, :])

            id2 = lambda h: h[:, :]
            row = lambda h: h[:].rearrange("(a f) -> a f", a=1)
            col = lambda h: h[:].rearrange("(f a) -> f a", a=1)
            tr2 = lambda h: h[:, :].rearrange("n k -> k n")

            sample("l1_whh", id2, H, G1, 0, W1A[0:H, :])
            sample("l1_wih", id2, I, G1, H, W1A[H:H + I, :])
            # b1 is sampled at a 32-aligned partition base (b1s), then
            # DMA-copied into its (unaligned) weight-tile rows AFTER
            # finish_samples() below.
            sample("l1_b", row, 1, G1, 0, b1s[0:1, :])
            sample("l1_whh", id2, H, G1, 64, W1hB[64:128, :])
            sample("l1_wih", id2, I, G1, 0, W1xB[0:I, :])
            sample("l2_whh", id2, H2, G2, 0, W2h[:, :])
            sample("l2_wih", id2, H, G2, 0, W2x[0:H, :])
            sample("l2_b", row, 1, G2, H, W2x[H:H + 1, :])
            sample("fc1_w", tr2, H2, N, 0, fc1wT[:, :])
            sample("fc2_w", tr2, N, N, 0, fc2wT[:, :])
            sample("out_w", tr2, N, 1, 0, outwT[:, :])
            sample("fc1_b", col, N, 1, 0, fc1b[:, :])
            sample("fc2_b", col, N, 1, 0, fc2b[:, :])
            sample("out_b", col, 1, 1, 0, outb[:, :])
            finish_samples()
            nc.sync.dma_start(out=W1A[H + I:H + I + 1, :], in_=b1s[0:1, :])
            nc.sync.dma_start(out=W1xB[I:I + 1, :], in_=b1s[0:1, :])

        # ---------------- pre-pass: xT = x.T via PE transposes --------------
        # keep all batch tiles resident; assemble whole [128, BC] row-blocks
        # in SBUF so each xT write is one big contiguous DMA.
        NBLK = (TIl + 127) // 128
        NBT = BC // 128
        with tc.tile_pool(name="xload", bufs=1) as xl, \
             tc.tile_pool(name="xst", bufs=2) as xs, \
             tc.tile_pool(name="xps", bufs=4, space="PSUM") as xp:
            xins = []
            for bt in range(NBT):
                xin = xl.tile([128, TIl], F32, tag=f"xin{bt}", name=f"xin{bt}")
                nc.sync.dma_start(
                    out=xin[:, :],
                    in_=x[bt * 128:(bt + 1) * 128, :, :].rearrange("b t i -> b (t i)"),
                )
                xins.append(xin)
            for blk in range(NBLK):
                w = min(128, TIl - blk * 128)
                stg = xs.tile([128, BC], BF16, tag="stg", name="stg")
                for bt in range(NBT):
                    ps = xp.tile([128, 128], F32, tag="tps", name="tps")
                    nc.tensor.transpose(
                        ps[0:w, 0:128],
                        xins[bt][:, blk * 128:blk * 128 + w], ident[:, :]
                    )
                    if bt % 2 == 0:
                        nc.vector.tensor_copy(
                            stg[0:w, bt * 128:(bt + 1) * 128], ps[0:w, :])
                    else:
                        nc.scalar.copy(
                            stg[0:w, bt * 128:(bt + 1) * 128], ps[0:w, :])
                nc.sync.dma_start(out=xT[blk * 128:blk * 128 + w, :],
                                  in_=stg[0:w, :])

        # -------- fused recurrence: L1 step u + L2 step u-1 per iteration ----
        # hxA: rows 0:64 h1(batch half A), 64:88 x_t, 88 ones  (rhs K=89 @ base 0)
        # hxB: rows 0:24 x_t, 24 ones, 64:128 h1(batch half B)
        # L2 runs one step behind L1; h1_t is copied (SBUF->SBUF DMA) into the
        # aux tiles ([h1; ones], K=65 rhs) the same iteration it is produced.
        hxA = [fixed([128, BH], f"hxA{k}", BF16) for k in range(2)]
        hxB = [fixed([128, BH], f"hxB{k}", BF16) for k in range(2)]
        c1t = fixed([128, BH], "c1t")
        ones_row = fixed([1, BH], "ones_row", BF16)
        h2 = [fixed([128, BH], f"h2_{ch}", BF16) for ch in range(2)]
        c2 = [fixed([128, BH], f"c2_{ch}") for ch in range(2)]
        aux = [[fixed([128, BH], f"aux{ch}_{k}", BF16) for k in range(2)]
               for ch in range(2)]
        nc.vector.memset(ones_row[:, :], 1.0)
        nc.vector.memset(c1t[:, :], 0.0)
        nc.vector.memset(hxA[0][0:H, :], 0.0)
        nc.vector.memset(hxB[0][64:128, :], 0.0)
        for k in range(2):
            # ones rows sit at unaligned partitions -> fill via DMA copy
            nc.sync.dma_start(out=hxA[k][H + I:H + I + 1, :], in_=ones_row[0:1, :])
            nc.sync.dma_start(out=hxB[k][I:I + 1, :], in_=ones_row[0:1, :])
        for ch in range(2):
            nc.vector.memset(h2[ch][:, :], 0.0)
            nc.vector.memset(c2[ch][:, :], 0.0)
            for k in range(2):
                nc.vector.memset(aux[ch][k][H:H + 1, :], 1.0)

        # (sigma-free-offset, weight-col-offset): i, f, o then g
        L1_SIG = [(0, 0), (BH, H), (2 * BH, 3 * H)]
        L1_G = 2 * H
        L2_SIG = [(0, 0), (BH, H2), (2 * BH, 3 * H2)]
        L2_G = 2 * H2

        with tc.tile_pool(name="p1ps", bufs=1, space="PSUM") as pps, \
             tc.tile_pool(name="p1sb", bufs=3) as psb, \
             tc.tile_pool(name="p2ps", bufs=1, space="PSUM") as pps2, \
             tc.tile_pool(name="p2sb", bufs=3) as psb2:

            def l1_step(t):
                cur, nxt = t % 2, (t + 1) % 2
                nc.sync.dma_start(out=hxA[cur][H:H + I, :],
                                  in_=xT[t * I:(t + 1) * I, 0:BH])
                nc.sync.dma_start(out=hxB[cur][0:I, :],
                                  in_=xT[t * I:(t + 1) * I, BH:BC])
                sps = pps.tile([128, 3 * BH], F32, tag="sps", name="sps")
                gps = pps.tile([128, BH], F32, tag="gps", name="gps")
                for fo, wc in L1_SIG + [(None, L1_G)]:
                    wsl = slice(wc, wc + H)
                    if fo is None:
                        outA, outB = gps[0:64, :], gps[64:128, :]
                    else:
                        outA = sps[0:64, fo:fo + BH]
                        outB = sps[64:128, fo:fo + BH]
                    nc.tensor.matmul(outA, lhsT=W1A[0:H + I + 1, wsl],
                                     rhs=hxA[cur][0:H + I + 1, :],
                                     start=True, stop=True)
                    nc.tensor.matmul(outB, lhsT=W1hB[64:128, wsl],
                                     rhs=hxB[cur][64:128, :],
                                     start=True, stop=False)
                    nc.tensor.matmul(outB, lhsT=W1xB[0:I + 1, wsl],
                                     rhs=hxB[cur][0:I + 1, :],
                                     start=False, stop=True)
                ssb = psb.tile([128, 3 * BH], F32, tag="ssb", name="ssb")
                tg = psb.tile([128, BH], F32, tag="tg", name="tg")
                tcn = psb.tile([128, BH], F32, tag="tcn", name="tcn")
                pp = psb.tile([128, BH], F32, tag="pp", name="pp")
                qq = psb.tile([128, BH], F32, tag="qq", name="qq")
                nc.scalar.activation(ssb[:, :], sps[:, :], AF.Sigmoid)
                nc.scalar.activation(tg[:, :], gps[:, :], AF.Tanh)
                nc.vector.tensor_mul(pp[:, :], ssb[:, BH:2 * BH], c1t[:, :])
                nc.gpsimd.tensor_mul(qq[:, :], ssb[:, 0:BH], tg[:, :])
                nc.vector.tensor_add(c1t[:, :], pp[:, :], qq[:, :])
                nc.scalar.activation(tcn[:, :], c1t[:, :], AF.Tanh)
                nc.vector.tensor_mul(hxA[nxt][0:H, :],
                                     ssb[0:H, 2 * BH:3 * BH], tcn[0:H, :])
                nc.vector.tensor_mul(hxB[nxt][64:128, :],
                                     ssb[64:128, 2 * BH:3 * BH], tcn[64:128, :])
                # hand h1_t to layer 2 (partition-shifting copies -> DMA)
                nc.sync.dma_start(out=aux[0][t % 2][0:H, :], in_=hxA[nxt][0:H, :])
                nc.sync.dma_start(out=aux[1][t % 2][0:H, :],
                                  in_=hxB[nxt][64:128, :])

            def l2_step(t):
                k = t % 2
                for ch in range(2):
                    sps = pps2.tile([128, 3 * BH], F32, tag="sps2", name="sps2")
                    gps = pps2.tile([128, BH], F32, tag="gps2", name="gps2")
                    for fo, wc in L2_SIG + [(None, L2_G)]:
                        wsl = slice(wc, wc + H2)
                        out = gps[:, :] if fo is None else sps[:, fo:fo + BH]
                        nc.tensor.matmul(out, lhsT=W2x[0:H + 1, wsl],
                                         rhs=aux[ch][k][0:H + 1, :],
                                         start=True, stop=False)
                        nc.tensor.matmul(out, lhsT=W2h[:, wsl],
                                         rhs=h2[ch][:, :],
                                         start=False, stop=True)
                    ssb = psb2.tile([128, 3 * BH], F32, tag="ssb2", name="ssb2")
                    tg = psb2.tile([128, BH], F32, tag="tg2", name="tg2")
                    tcn = psb2.tile([128, BH], F32, tag="tcn2", name="tcn2")
                    pp = psb2.tile([128, BH], F32, tag="pp2", name="pp2")
                    qq = psb2.tile([128, BH], F32, tag="qq2", name="qq2")
                    nc.scalar.activation(ssb[:, :], sps[:, :], AF.Sigmoid)
                    nc.scalar.activation(tg[:, :], gps[:, :], AF.Tanh)
                    nc.vector.tensor_mul(pp[:, :], ssb[:, BH:2 * BH], c2[ch][:, :])
                    nc.gpsimd.tensor_mul(qq[:, :], ssb[:, 0:BH], tg[:, :])
                    nc.vector.tensor_add(c2[ch][:, :], pp[:, :], qq[:, :])
                    nc.scalar.activation(tcn[:, :], c2[ch][:, :], AF.Tanh)
                    nc.vector.tensor_mul(h2[ch][:, :],
                                         ssb[:, 2 * BH:3 * BH], tcn[:, :])

            for u in range(t_steps + 1):
                if u < t_steps:
                    l1_step(u)
                if u >= 1:
                    l2_step(u - 1)

        # ---------------- head: fc1 -> relu -> fc2 -> relu -> out -----------
        with tc.tile_pool(name="hps", bufs=2, space="PSUM") as hps, \
             tc.tile_pool(name="hsb", bufs=2) as hsb:
            for ch in range(2):
                f1 = hps.tile([N, BH], F32, tag="f1", name="f1")
                nc.tensor.matmul(f1[0:N, :], lhsT=fc1wT[0:H2, 0:N],
                                 rhs=h2[ch][:, :], start=True, stop=True)
                x1 = hsb.tile([N, BH], BF16, tag="x1", name="x1")
                nc.scalar.activation(x1[0:N, :], f1[0:N, :], AF.Relu,
                                     bias=fc1b[:, :])
                f2 = hps.tile([N, BH], F32, tag="f2", name="f2")
                nc.tensor.matmul(f2[0:N, :], lhsT=fc2wT[0:N, 0:N],
                                 rhs=x1[0:N, :], start=True, stop=True)
                x2 = hsb.tile([N, BH], BF16, tag="x2", name="x2")
                nc.scalar.activation(x2[0:N, :], f2[0:N, :], AF.Relu,
                                     bias=fc2b[:, :])
                fy = hps.tile([1, BH], F32, tag="fy", name="fy")
                nc.tensor.matmul(fy[0:1, :], lhsT=outwT[0:N, 0:1],
                                 rhs=x2[0:N, :], start=True, stop=True)
                ysb = hsb.tile([1, BH], F32, tag="ysb", name="ysb")
                nc.scalar.activation(ysb[0:1, :], fy[0:1, :], AF.Identity,
                                     bias=outb[:, :])
                nc.sync.dma_start(
                    out=y[ch * BH:(ch + 1) * BH].rearrange("(a f) -> a f", a=1),
                    in_=ysb[0:1, :],
                )

        # release single-tile pools in LIFO order so no pool-boundary
        # pseudo-instructions survive into the lowered BIR
        for free in reversed(_frees):
            free()

    # run the bacc legalization pipeline (sync-wait splitting, reg alloc, ...)
    nc.finalize()
    return nc


def run(inputs, trace=False):
    """Returns (y_full [8192] f32, BassKernelResults)."""
    xfull = np.ascontiguousarray(np.asarray(inputs["input_seq"], dtype=np.float32))
    base = {}
    for name, _shape in PARAMS:
        for sfx in ("mu", "rho", "eps"):
            n = f"{name}_{sfx}"
            base[n] = np.ascontiguousarray(np.asarray(inputs[n], dtype=np.float32))
    in_maps = []
    for c in range(NCORES):
        m = dict(base)
        m["x"] = np.ascontiguousarray(xfull[c * BC:(c + 1) * BC])
        in_maps.append(m)
    nc = _build()
    res = run_bass_kernel_spmd(nc, in_maps, core_ids=list(range(NCORES)),
                               trace=trace)
    out = np.concatenate([r["y"] for r in res.results]).astype(np.float32)
    return out, res


def kernel(**inputs):
    out, _ = run(inputs, trace=False)
    return out
